# revision 1
# baseline (speedup 1.0000x reference)
"""Biaffine (trilinear + concat-linear) kernel for Trainium2, 8-core SPMD.

logits[b,x,y,o] = sum_ij in1[b,x,i] * w1[i,o,j] * in2[b,y,j]
               + termA[b,x,o] + termB[b,y,o] + bias[o]
  termA[b,x,o] = sum_i in1[b,x,i] * w2[i,o]
  termB[b,y,o] = sum_j in1[b,y,j] * w2[IN+j,o]   (both halves from input1!)
  bias[o]      = w2[2*IN,o]

Sharding: core c handles batch b=c//2, x-range [x0, x0+256), x0=256*(c%2).
w1/w2 replicated. Per core, two chained matmul phases over o-chunks of OC
(w1 is streamed through SBUF exactly once per core, batched OB o's per DMA,
host pre-casts it to bf16 to halve HBM traffic):
  phase 1: temp[j, o, x] = sum_i w1[i,o,j] * in1[x,i]
           (stationary = w1 128x128 tile, moving = in1^T [128, 256], fp32
           PSUM accumulation over 4 i-blocks, result stored bf16)
  phase 2: out[x, y] (per o) = sum_jblk temp-slice[j, x] @ in2T[j, y]
           + selector-matmul: lhsT[k,m] = identw[k,o] (free-broadcast AP)
             x rhs TBb[k,y]  ==> adds termB[y,o]+bias[o] to every x row
           then PSUM->SBUF drain + termA[x,o] bias-add on the ACT engine
           (scalar.activation Identity with per-partition bias) -- keeping
           DVE free for the phase-1 copies; DVE per-op DRAIN overhead made
           DVE the critical path when it carried both streams
temp is double-buffered so phase 1 of chunk N+1 overlaps phase 2 of chunk N.
Device output layout [x, o, y] so every output DMA line is >=14KB
contiguous; the host transposes to [x, y, o] while unsharding.

The selector matmul MUST use the bf16 identity (identw): with an fp32
zero-stride broadcast stationary the weight load takes a pathological slow
path and the whole main loop nearly doubles (measured ~0.96 ms -> ~0.6 ms
per core after switching it to bf16). Moving the phase-2 PSUM drain from
DVE to ACT measured ~2.4-2.8 ms faster over 6 chained iterations in a
same-window interleaved R6-vs-R6 A/B (bench3.py) — the most reliable
differential this tunnel allows. Absolute main-loop estimates across
measurement windows span ~0.5-0.75 ms/core (window-dependent bias of the
bimodal ~37/78 ms tunnel round-trip); cost-model TimelineSim says 0.47 ms.
jb_pack=True (single DVE drain for two packed PSUM groups) measured
neutral-to-worse; left off.
"""

import numpy as np

B, S, IN, OUT = 4, 512, 512, 112
N_CORES = 8
P = 128


def split_sync_waits(nc, max_waits=1):
    """The walrus codegen in this toolchain rejects instructions carrying
    more than a few semaphore waits ("Too many sync wait commands").
    Hoist overflow waits onto NoOps inserted just before the instruction,
    on the same engine (semantically identical: the sequencer blocks on
    each wait in order)."""
    import concourse.mybir as mybir

    n_split = 0
    for f in nc.m.functions:
        for bb in f.blocks:
            new_insts = []
            for inst in bb.instructions:
                si = inst.sync_info
                if si is not None and si.on_wait and len(si.on_wait) > max_waits:
                    waits = list(si.on_wait)
                    overflow, keep = waits[:-max_waits], waits[-max_waits:]
                    for k in range(0, len(overflow), max_waits):
                        chunk = overflow[k:k + max_waits]
                        nop = mybir.InstNoOp(
                            name=f"{inst.name}_wsplit{k}",
                            opcode="NoOp",
                            engine=inst.engine,
                            sync_info=mybir.SyncInfo(on_wait=chunk, on_update=[]),
                        )
                        new_insts.append(nop)
                        n_split += 1
                    si.on_wait = keep
                new_insts.append(inst)
            bb.instructions[:] = new_insts
    return n_split


def build_nc(S_=S, IN_=IN, OUT_=OUT, XW=256, OC=14, OG=7, OB=7, w1_bf16=True,
             temp_bufs=2, split_waits=True, repeat=1, only_phase=0,
             act_drain=True, jb_pack=False):
    """Build the per-core Bass module. All 8 cores run the same program on
    their own input slices (SPMD)."""
    import concourse.bass as bass
    import concourse.mybir as mybir
    import concourse.tile as tile
    from concourse.masks import make_identity

    f32 = mybir.dt.float32
    wdt = mybir.dt.bfloat16 if w1_bf16 else f32

    KI = IN_ // P          # number of 128-blocks of the i/j contraction dims
    YB = S_ // P           # y 128-blocks
    XB = XW // P           # x 128-blocks per core
    NCH = OUT_ // OC       # o-chunks
    assert OC % OG == 0 and OC % OB == 0

    nc = bass.Bass()
    in1x = nc.dram_tensor("in1x", [XW, IN_], f32, kind="ExternalInput")
    in1f = nc.dram_tensor("in1f", [S_, IN_], f32, kind="ExternalInput")
    in2f = nc.dram_tensor("in2f", [S_, IN_], f32, kind="ExternalInput")
    w1 = nc.dram_tensor("w1", [IN_, OUT_, IN_], wdt, kind="ExternalInput")
    w2 = nc.dram_tensor("w2", [2 * IN_ + 1, OUT_], f32, kind="ExternalInput")
    outp = nc.dram_tensor("outp", [XW, OUT_, S_], f32, kind="ExternalOutput")

    with tile.TileContext(nc) as tc:
        with tc.tile_pool(name="persist", bufs=1) as pers:
            # persistent SBUF tensors
            in1Tx = pers.tile([P, KI, XW], f32, name="in1Tx")   # in1x^T
            in1Tf = pers.tile([P, KI, S_], f32, name="in1Tf")   # in1f^T
            in2T = pers.tile([P, KI, S_], wdt, name="in2T")     # in2f^T
            wA = pers.tile([P, KI, OUT_], f32, name="wA")
            wB = pers.tile([P, KI, OUT_], f32, name="wB")
            biasc = pers.tile([OUT_, 1], f32, name="biasc")
            TBb = pers.tile([OUT_, S_], wdt, name="TBb")        # termB[y,o]+bias
            termA = pers.tile([P, XB, OUT_], f32, name="termA")
            ident = pers.tile([P, P], f32, name="ident")
            identw = pers.tile([P, P], wdt, name="identw")
            if w1_bf16:
                in1Tx_b = pers.tile([P, KI, XW], wdt, name="in1Tx_b")

            # ---------------- prep: transposes + affine terms ----------------
            with tc.tile_pool(name="prep", bufs=2) as prep, \
                 tc.tile_pool(name="prep_ps", bufs=2, space="PSUM") as prep_ps:
                make_identity(nc, ident)
                nc.vector.tensor_copy(identw, ident)

                nc.sync.dma_start(wA, w2[0:IN_, :].rearrange("(a p) o -> p a o", p=P))
                nc.sync.dma_start(wB, w2[IN_:2 * IN_, :].rearrange("(a p) o -> p a o", p=P))
                with nc.allow_non_contiguous_dma(reason="112B one-time bias load"):
                    nc.sync.dma_start(biasc, w2[2 * IN_:2 * IN_ + 1, :].rearrange("a o -> o a"))

                def transpose_into(dst, src_dram, rows):
                    # src_dram: [rows, IN_] fp32 -> dst [P, KI, rows] (= src^T)
                    st = prep.tile([P, rows // P, IN_], f32, name="stage", tag="stage")
                    nc.sync.dma_start(st, src_dram[:, :].rearrange("(a p) i -> p a i", p=P))
                    for a in range(rows // P):
                        for ib in range(KI):
                            pt = prep_ps.tile([P, P], f32, name="pt", tag="pt")
                            nc.tensor.transpose(pt, st[:, a, ib * P:(ib + 1) * P], ident)
                            nc.vector.tensor_copy(dst[:, ib, a * P:(a + 1) * P], pt)

                transpose_into(in1Tx, in1x, XW)
                transpose_into(in1Tf, in1f, S_)
                transpose_into(in2T, in2f, S_)  # cast to wdt in the copy
                if w1_bf16:
                    nc.vector.tensor_copy(in1Tx_b, in1Tx)

                # TBb[o, y] = sum_j wB[j,o] * in1f[y,j] + bias[o]
                psTB = prep_ps.tile([OUT_, S_], f32, name="psTB", tag="psTB")
                for jb in range(KI):
                    nc.tensor.matmul(psTB, wB[:, jb, :], in1Tf[:, jb, :],
                                     start=(jb == 0), stop=(jb == KI - 1))
                nc.vector.tensor_scalar_add(TBb, psTB, biasc)

                # termA[x, o] = sum_i in1x[x,i] * wA[i,o]
                for xb in range(XB):
                    psA = prep_ps.tile([P, OUT_], f32, name="psA", tag="psA")
                    for ib in range(KI):
                        nc.tensor.matmul(psA, in1Tx[:, ib, xb * P:(xb + 1) * P],
                                         wA[:, ib, :],
                                         start=(ib == 0), stop=(ib == KI - 1))
                    nc.vector.tensor_copy(termA[:, xb, :], psA)

            # ---------------- main: o-chunked two-phase pipeline ----------------
            with tc.tile_pool(name="w1p", bufs=8) as w1p, \
                 tc.tile_pool(name="tempp", bufs=temp_bufs) as tempp, \
                 tc.tile_pool(name="outsb", bufs=3) as outsb, \
                 tc.tile_pool(name="ps1", bufs=4, space="PSUM") as ps1p, \
                 tc.tile_pool(name="ps2", bufs=4, space="PSUM") as ps2p:
                rhs1 = in1Tx_b if w1_bf16 else in1Tx
                for oc in [c for _ in range(repeat) for c in range(NCH)]:
                    # phase 1: temp[j, ol, x] for this o-chunk
                    temp = tempp.tile([P, KI, OC, XW], wdt, name="temp", tag="temp")
                    for og in range(OC // OB) if only_phase in (0, 1) else []:
                        w1t = []
                        for ib in range(KI):
                            t = w1p.tile([P, OB, IN_], wdt, name="w1t", tag="w1t")
                            nc.sync.dma_start(
                                t, w1[ib * P:(ib + 1) * P,
                                      oc * OC + og * OB:oc * OC + (og + 1) * OB, :])
                            w1t.append(t)
                        for bl in range(OB):
                            ol = og * OB + bl
                            if jb_pack:
                                # two j-block accumulation groups share one
                                # PSUM bank (disjoint column halves) so ONE
                                # DVE copy drains both -- halves the DVE op
                                # count (per-op DRAIN overhead dominates DVE)
                                for jp in range(KI // 2):
                                    ps1 = ps1p.tile([P, 2, XW], f32,
                                                    name="ps1", tag="ps1")
                                    for h in range(2):
                                        jb = 2 * jp + h
                                        for ib in range(KI):
                                            nc.tensor.matmul(
                                                ps1[:, h, :],
                                                w1t[ib][:, bl, jb * P:(jb + 1) * P],
                                                rhs1[:, ib, :],
                                                start=(ib == 0),
                                                stop=(ib == KI - 1))
                                    nc.vector.tensor_copy(
                                        temp[:, 2 * jp:2 * jp + 2, ol, :], ps1)
                            else:
                                for jb in range(KI):
                                    ps1 = ps1p.tile([P, XW], f32, name="ps1", tag="ps1")
                                    for ib in range(KI):
                                        nc.tensor.matmul(
                                            ps1, w1t[ib][:, bl, jb * P:(jb + 1) * P],
                                            rhs1[:, ib, :],
                                            start=(ib == 0), stop=(ib == KI - 1))
                                    nc.vector.tensor_copy(temp[:, jb, ol, :], ps1)
                    # phase 2: out[x, y] per o, + affine
                    for xb in range(XB) if only_phase in (0, 2) else []:
                        for g in range(OC // OG):
                            ot = outsb.tile([P, OG, S_], f32, name="ot", tag="ot")
                            for gl in range(OG):
                                ol = g * OG + gl
                                o = oc * OC + ol
                                ps2 = ps2p.tile([P, S_], f32, name="ps2", tag="ps2")
                                # selector matmul adds TBb[o, :] to every x row:
                                # lhsT[k, m] = ident[k, o] (free-broadcast), so
                                # out[m, n] += sum_k ident[k,o] * TBb[k,n] = TBb[o,n]
                                nc.tensor.matmul(
                                    ps2,
                                    identw[0:OUT_, o:o + 1].to_broadcast((OUT_, P)),
                                    TBb,
                                    start=True, stop=False)
                                for jb in range(KI):
                                    nc.tensor.matmul(
                                        ps2, temp[:, jb, ol, xb * P:(xb + 1) * P],
                                        in2T[:, jb, :],
                                        start=False, stop=(jb == KI - 1))
                                if act_drain:
                                    # PSUM drain + termA add on the idle ACT
                                    # engine: out = Identity(in*1 + bias)
                                    nc.scalar.activation(
                                        ot[:, gl, :], ps2,
                                        mybir.ActivationFunctionType.Identity,
                                        bias=termA[:, xb, o:o + 1])
                                else:
                                    nc.vector.tensor_scalar_add(
                                        ot[:, gl, :], ps2,
                                        termA[:, xb, o:o + 1])
                            nc.sync.dma_start(
                                outp[xb * P:(xb + 1) * P,
                                     oc * OC + g * OG:oc * OC + (g + 1) * OG, :],
                                ot)

    if split_waits:
        split_sync_waits(nc)
    return nc


_CACHE = {}


def _get_nc(**kw):
    key = tuple(sorted(kw.items()))
    if key not in _CACHE:
        _CACHE[key] = build_nc(**kw)
    return _CACHE[key]


W1_BF16 = True
TRACE = False
LAST_RESULT = None


def kernel(input1, input2, w1, w2, seq_len=None, **_ignored):
    global LAST_RESULT
    from concourse.bass_utils import run_bass_kernel_spmd
    import ml_dtypes

    input1 = np.asarray(input1, dtype=np.float32)
    input2 = np.asarray(input2, dtype=np.float32)
    w1 = np.asarray(w1, dtype=np.float32)
    w2 = np.asarray(w2, dtype=np.float32)

    nc = _get_nc(w1_bf16=W1_BF16)
    w1_dev = w1.astype(ml_dtypes.bfloat16) if W1_BF16 else w1

    XW = S // 2
    in_maps = []
    for c in range(N_CORES):
        b, xh = divmod(c, 2)
        x0 = xh * XW
        in_maps.append({
            "in1x": np.ascontiguousarray(input1[b, x0:x0 + XW, :]),
            "in1f": input1[b],
            "in2f": input2[b],
            "w1": w1_dev,
            "w2": w2,
        })
    res = run_bass_kernel_spmd(nc, in_maps, core_ids=list(range(N_CORES)),
                               trace=TRACE)
    LAST_RESULT = res

    full = np.empty((B, S, S, OUT), dtype=np.float32)
    for c in range(N_CORES):
        b, xh = divmod(c, 2)
        x0 = xh * XW
        # device layout [x, o, y] -> [x, y, o]
        full[b, x0:x0 + XW] = res.results[c]["outp"].transpose(0, 2, 1)
    return full



# revision 11
# speedup vs baseline: 1.1314x; 1.1314x over previous
"""Biaffine (trilinear + concat-linear) kernel for Trainium2, 8-core SPMD.

logits[b,x,y,o] = sum_ij in1[b,x,i] * w1[i,o,j] * in2[b,y,j]
               + termA[b,x,o] + termB[b,y,o] + bias[o]
  termA[b,x,o] = sum_i in1[b,x,i] * w2[i,o]
  termB[b,y,o] = sum_j in1[b,y,j] * w2[IN+j,o]   (both halves from input1!)
  bias[o]      = w2[2*IN,o]

Sharding: OUTPUT-dim sharding. Core c owns o in [14c, 14c+14), ALL batches
and the full S x S plane. This cuts per-core w1 HBM traffic 8x vs
batch/x sharding (7.3 MB bf16 instead of 58.7 MB) and lets both matmul
phases batch their moving operand over the batch dim, so each stationary
(weight) load streams 1024-2048 columns instead of 256 -> far fewer
weight loads (the dominant un-modeled HW cost) and fewer instructions.

Host-side prep (cheap, O(S*IN) or O(S*OUT) only):
  - in1T/in2T: inputs pre-transposed to [IN, B, S] and cast to bf16
    (kills all on-device PE transposes of the baseline).
  - termA/termB+bias: 60 MFLOP of affine matmuls (0.025% of total work)
    computed in numpy; termA is uploaded pre-replicated across the 128
    partitions so the device can add it along the free axis with a plain
    DVE op (a partition-stride-0 read is impossible for DVE; the
    baseline burned PE columns on a "selector matmul" for this instead).

Device, per o-pair chunk (7 chunks of OC=2):
  phase 1: temp[j, b, o, x] = sum_i w1[i,o,j] * in1T[i,(b,x)]
           stationary = w1 128x128 tile (reused for 2 batch-matmuls),
           moving = in1T [128, 512], fp32 PSUM accumulate over 4
           i-blocks, drained fp32->bf16 on the ACT engine.
  phase 2: out[y, (o,x)] = sum_jb in2T-tile^T @ temp-tile
           stationary = in2T 128x128 tile (reused for 2 o-matmuls),
           moving = temp [128, 512]; drain is ONE fused DVE
           scalar_tensor_tensor per (o): out = (psum + termB[y,o]) +
           termA_rep[o,x] -- both affine terms applied in a single pass,
           nothing but matmuls ever touches the PE.
temp is double-buffered so phase 1 of chunk N+1 overlaps phase 2 of N.
PSUM: phase-1 pool 2x[128,2,512] (4 banks) + phase-2 pool 2x (4 banks).
Device output layout [b, y, o_local, x] (4 KB contiguous DMA lines);
host transposes to [b, x, y, o] while unsharding.
"""

import numpy as np

B, S, IN, OUT = 4, 512, 512, 112
N_CORES = 8
P = 128
OC = 2                    # o's per chunk (o-pair)
OPC = OUT // N_CORES      # 14 o's per core
NCH = OPC // OC           # 7 chunks


def split_sync_waits(nc, max_waits=1):
    """The walrus codegen in this toolchain rejects instructions carrying
    more than a few semaphore waits ("Too many sync wait commands").
    Hoist overflow waits onto NoOps inserted just before the instruction,
    on the same engine (semantically identical: the sequencer blocks on
    each wait in order)."""
    import concourse.mybir as mybir

    n_split = 0
    for f in nc.m.functions:
        for bb in f.blocks:
            new_insts = []
            for inst in bb.instructions:
                si = inst.sync_info
                if si is not None and si.on_wait and len(si.on_wait) > max_waits:
                    waits = list(si.on_wait)
                    overflow, keep = waits[:-max_waits], waits[-max_waits:]
                    for k in range(0, len(overflow), max_waits):
                        chunk = overflow[k:k + max_waits]
                        nop = mybir.InstNoOp(
                            name=f"{inst.name}_wsplit{k}",
                            opcode="NoOp",
                            engine=inst.engine,
                            sync_info=mybir.SyncInfo(on_wait=chunk, on_update=[]),
                        )
                        new_insts.append(nop)
                        n_split += 1
                    si.on_wait = keep
                new_insts.append(inst)
            bb.instructions[:] = new_insts
    return n_split


def build_nc(temp_bufs=2, split_waits=True, only_phase=0):
    """Build the per-core Bass module. All 8 cores run the same program on
    their own w1/termA/termB o-slices (SPMD)."""
    import concourse.bass as bass
    import concourse.mybir as mybir
    import concourse.tile as tile

    f32 = mybir.dt.float32
    bf16 = mybir.dt.bfloat16
    ADD = mybir.AluOpType.add
    COPY = mybir.ActivationFunctionType.Copy

    KI = IN // P   # 4 contraction blocks (i and j)
    YB = S // P    # 4 y blocks

    nc = bass.Bass()
    in1T = nc.dram_tensor("in1T", [IN, B, S], bf16, kind="ExternalInput")
    in2T = nc.dram_tensor("in2T", [IN, B, S], bf16, kind="ExternalInput")
    w1 = nc.dram_tensor("w1", [IN, OPC, IN], bf16, kind="ExternalInput")
    tArep = nc.dram_tensor("tArep", [P, B, OPC, S], f32, kind="ExternalInput")
    tBbT = nc.dram_tensor("tBbT", [P, B, YB, OPC], f32, kind="ExternalInput")
    outp = nc.dram_tensor("outp", [B, S, OPC, S], f32, kind="ExternalOutput")

    with tile.TileContext(nc) as tc:
        with tc.tile_pool(name="persist", bufs=1) as pers:
            in1Ts = pers.tile([P, KI, B, S], bf16, name="in1Ts")
            in2Ts = pers.tile([P, KI, B, S], bf16, name="in2Ts")
            tBs = pers.tile([P, B, YB, OPC], f32, name="tBs")
            nc.sync.dma_start(in1Ts, in1T.rearrange("(a p) b x -> p a b x", p=P))
            nc.sync.dma_start(in2Ts, in2T.rearrange("(a p) b y -> p a b y", p=P))
            nc.sync.dma_start(tBs, tBbT[:, :, :, :])

            with tc.tile_pool(name="w1p", bufs=2 * OC) as w1p, \
                 tc.tile_pool(name="tempp", bufs=temp_bufs) as tempp, \
                 tc.tile_pool(name="repp", bufs=2 * B) as repp, \
                 tc.tile_pool(name="otp", bufs=3) as otp, \
                 tc.tile_pool(name="ps1", bufs=2, space="PSUM") as ps1p, \
                 tc.tile_pool(name="ps2", bufs=4, space="PSUM") as ps2p:
                def stream_chunk_inputs(c):
                    # w1 slices + replicated termA for chunk c
                    w1t = []
                    for oo in range(OC):
                        t = w1p.tile([P, KI, IN], bf16, name="w1t", tag="w1t")
                        nc.sync.dma_start(
                            t, w1[:, c * OC + oo, :].rearrange("(a p) j -> p a j", p=P))
                        w1t.append(t)
                    rept = []
                    for b in range(B):
                        r = repp.tile([P, OC, S], f32, name="rep", tag="rep")
                        nc.sync.dma_start(r, tArep[:, b, c * OC:(c + 1) * OC, :])
                        rept.append(r)
                    return w1t, rept

                nxt = stream_chunk_inputs(0)
                for c in range(NCH):
                    w1t, rept = nxt

                    # ---- phase 1: temp[j, b, o, x] for this o-pair
                    temp = tempp.tile([P, KI, B, OC, S], bf16, name="temp", tag="temp")
                    for oo in range(OC) if only_phase in (0, 1) else []:
                        for jb in range(KI):
                            for h in range(2):
                                ps = ps1p.tile([P, 2, S], f32, name="ps1", tag="ps1")
                                for ib in range(KI):
                                    lhsT = w1t[oo][:, ib, jb * P:(jb + 1) * P]
                                    nc.tensor.matmul(
                                        ps[:, 0, :], lhsT, in1Ts[:, ib, 2 * h, :],
                                        start=(ib == 0), stop=(ib == KI - 1))
                                    nc.tensor.matmul(
                                        ps[:, 1, :], lhsT, in1Ts[:, ib, 2 * h + 1, :],
                                        start=(ib == 0), stop=(ib == KI - 1))
                                nc.scalar.activation(
                                    temp[:, jb, 2 * h:2 * h + 2, oo, :], ps, COPY)

                    # prefetch next chunk's inputs before phase 2 is emitted so
                    # its DMAs aren't queued behind this chunk's output stores
                    if c + 1 < NCH:
                        nxt = stream_chunk_inputs(c + 1)

                    # ---- phase 2: out[y, (o, x)] + affine, per (b, yblock)
                    # per-o single-bank psum tiles so each is freed after ONE
                    # drain op (a shared [128,2,512] tile was held ~2.0us by
                    # two sequential drains vs the 1.7us PE group period)
                    for b in range(B) if only_phase in (0, 2) else []:
                        for yb in range(YB):
                            pso = [ps2p.tile([P, S], f32, name="ps2", tag="ps2")
                                   for _ in range(OC)]
                            for jb in range(KI):
                                lhsT = in2Ts[:, jb, b, yb * P:(yb + 1) * P]
                                for oo in range(OC):
                                    nc.tensor.matmul(
                                        pso[oo], lhsT, temp[:, jb, b, oo, :],
                                        start=(jb == 0), stop=(jb == KI - 1))
                            ot = otp.tile([P, OC, S], f32, name="ot", tag="ot")
                            for oo in range(OC):
                                # out = (psum + termB[y,o]) + termA_rep[o, x]
                                o = c * OC + oo
                                nc.vector.scalar_tensor_tensor(
                                    ot[:, oo, :], pso[oo],
                                    tBs[:, b, yb, o:o + 1],
                                    rept[b][:, oo, :], ADD, ADD)
                            nc.sync.dma_start(
                                outp[b, yb * P:(yb + 1) * P,
                                     c * OC:(c + 1) * OC, :], ot)

    if split_waits:
        split_sync_waits(nc)
    return nc


_CACHE = {}


def _get_nc(**kw):
    key = tuple(sorted(kw.items()))
    if key not in _CACHE:
        _CACHE[key] = build_nc(**kw)
    return _CACHE[key]


TRACE = False
LAST_RESULT = None


def kernel(input1, input2, w1, w2, seq_len=None, **_ignored):
    global LAST_RESULT
    from concourse.bass_utils import run_bass_kernel_spmd
    import ml_dtypes

    bf16 = ml_dtypes.bfloat16
    input1 = np.asarray(input1, dtype=np.float32)
    input2 = np.asarray(input2, dtype=np.float32)
    w1 = np.asarray(w1, dtype=np.float32)
    w2 = np.asarray(w2, dtype=np.float32)

    nc = _get_nc()

    # host-side layout prep (cheap): transposed bf16 inputs, affine terms
    in1T = np.ascontiguousarray(input1.transpose(2, 0, 1)).astype(bf16)  # [IN,B,S]
    in2T = np.ascontiguousarray(input2.transpose(2, 0, 1)).astype(bf16)
    wA, wB, bias = w2[:IN], w2[IN:2 * IN], w2[2 * IN]
    termA = np.einsum('bxi,io->box', input1, wA)            # [B, OUT, S]
    termB = input1 @ wB + bias                              # [B, S, OUT]

    in_maps = []
    for c in range(N_CORES):
        o0 = c * OPC
        w1c = np.ascontiguousarray(w1[:, o0:o0 + OPC, :]).astype(bf16)
        tA = np.ascontiguousarray(termA[:, o0:o0 + OPC, :], dtype=np.float32)
        tArep = np.ascontiguousarray(
            np.broadcast_to(tA[None], (P, B, OPC, S)))      # [128,B,OPC,S]
        # tBbT[p, b, yb, o] = termB[b, yb*128+p, o0+o]
        tBbT = np.ascontiguousarray(
            termB[:, :, o0:o0 + OPC].reshape(B, S // P, P, OPC)
            .transpose(2, 0, 1, 3), dtype=np.float32)
        in_maps.append({
            "in1T": in1T,
            "in2T": in2T,
            "w1": w1c,
            "tArep": tArep,
            "tBbT": tBbT,
        })
    res = run_bass_kernel_spmd(nc, in_maps, core_ids=list(range(N_CORES)),
                               trace=TRACE)
    LAST_RESULT = res

    full = np.empty((B, S, S, OUT), dtype=np.float32)
    for c in range(N_CORES):
        o0 = c * OPC
        # device layout [b, y, o, x] -> [b, x, y, o]
        full[:, :, :, o0:o0 + OPC] = res.results[c]["outp"].transpose(0, 3, 1, 2)
    return full


# revision 21
# speedup vs baseline: 1.1940x; 1.0553x over previous
"""Biaffine (trilinear + concat-linear) kernel for Trainium2, 8-core SPMD.

logits[b,x,y,o] = sum_ij in1[b,x,i] * w1[i,o,j] * in2[b,y,j]
               + termA[b,x,o] + termB[b,y,o] + bias[o]
  termA[b,x,o] = sum_i in1[b,x,i] * w2[i,o]
  termB[b,y,o] = sum_j in1[b,y,j] * w2[IN+j,o]   (both halves from input1!)
  bias[o]      = w2[2*IN,o]

Sharding: OUTPUT-dim sharding. Core c owns o in [14c, 14c+14), ALL batches
and the full S x S plane. This cuts per-core w1 HBM traffic 8x vs
batch/x sharding (7.3 MB bf16 instead of 58.7 MB) and lets both matmul
phases batch their moving operand over the batch dim, so each stationary
(weight) load streams 1024-2048 columns instead of 256 -> far fewer
weight loads (the dominant un-modeled HW cost) and fewer instructions.

Host-side prep (cheap, O(S*IN) or O(S*OUT) only):
  - in1T/in2T: inputs pre-transposed to [IN, B, S] and cast to bf16
    (kills all on-device PE transposes of the baseline).
  - termA/termB+bias: 60 MFLOP of affine matmuls (0.025% of total work)
    computed in numpy; termA is uploaded pre-replicated across the 128
    partitions so the device can add it along the free axis with a plain
    DVE op (a partition-stride-0 read is impossible for DVE; the
    baseline burned PE columns on a "selector matmul" for this instead).

Device, per o-pair chunk (7 chunks of OC=2):
  phase 1: temp[j, b, o, x] = sum_i w1[i,o,j] * in1T[i,(b,x)]
           stationary = w1 128x128 tile (reused for 2 batch-matmuls),
           moving = in1T [128, 512], fp32 PSUM accumulate over 4
           i-blocks, drained fp32->bf16 on the ACT engine.
  phase 2: out[y, (o,x)] = sum_jb in2T-tile^T @ temp-tile
           stationary = in2T 128x128 tile (reused for 2 o-matmuls),
           moving = temp [128, 512]; drain is ONE fused DVE
           scalar_tensor_tensor per (o): out = (psum + termB[y,o]) +
           termA_rep[o,x] -- both affine terms applied in a single pass,
           nothing but matmuls ever touches the PE.
temp is double-buffered so phase 1 of chunk N+1 overlaps phase 2 of N.
PSUM: phase-1 pool 2x[128,2,512] (4 banks) + phase-2 pool 2x (4 banks).
Device output layout [b, y, o_local, x] (4 KB contiguous DMA lines);
host transposes to [b, x, y, o] while unsharding.
"""

import numpy as np

B, S, IN, OUT = 4, 512, 512, 112
N_CORES = 8
P = 128
OC = 2                    # o's per chunk (o-pair)
OPC = OUT // N_CORES      # 14 o's per core
NCH = OPC // OC           # 7 chunks


def split_sync_waits(nc, max_waits=1):
    """The walrus codegen in this toolchain rejects instructions carrying
    more than a few semaphore waits ("Too many sync wait commands").
    Hoist overflow waits onto NoOps inserted just before the instruction,
    on the same engine (semantically identical: the sequencer blocks on
    each wait in order)."""
    import concourse.mybir as mybir

    n_split = 0
    for f in nc.m.functions:
        for bb in f.blocks:
            new_insts = []
            for inst in bb.instructions:
                si = inst.sync_info
                if si is not None and si.on_wait and len(si.on_wait) > max_waits:
                    waits = list(si.on_wait)
                    overflow, keep = waits[:-max_waits], waits[-max_waits:]
                    for k in range(0, len(overflow), max_waits):
                        chunk = overflow[k:k + max_waits]
                        nop = mybir.InstNoOp(
                            name=f"{inst.name}_wsplit{k}",
                            opcode="NoOp",
                            engine=inst.engine,
                            sync_info=mybir.SyncInfo(on_wait=chunk, on_update=[]),
                        )
                        new_insts.append(nop)
                        n_split += 1
                    si.on_wait = keep
                new_insts.append(inst)
            bb.instructions[:] = new_insts
    return n_split


def build_nc(temp_bufs=2, split_waits=True, only_phase=0):
    """Build the per-core Bass module. All 8 cores run the same program on
    their own w1/termA/termB o-slices (SPMD)."""
    import concourse.bass as bass
    import concourse.mybir as mybir
    import concourse.tile as tile

    f32 = mybir.dt.float32
    bf16 = mybir.dt.bfloat16
    ADD = mybir.AluOpType.add
    COPY = mybir.ActivationFunctionType.Copy

    KI = IN // P   # 4 contraction blocks (i and j)
    YB = S // P    # 4 y blocks

    nc = bass.Bass()
    in1T = nc.dram_tensor("in1T", [IN, B, S], bf16, kind="ExternalInput")
    in2T = nc.dram_tensor("in2T", [IN, B, S], bf16, kind="ExternalInput")
    w1 = nc.dram_tensor("w1", [IN, OPC, IN], bf16, kind="ExternalInput")
    tArep = nc.dram_tensor("tArep", [P, B, OPC, S], f32, kind="ExternalInput")
    tBbT = nc.dram_tensor("tBbT", [P, B, YB, OPC], f32, kind="ExternalInput")
    outp = nc.dram_tensor("outp", [B, S, OPC, S], f32, kind="ExternalOutput")

    with tile.TileContext(nc) as tc:
        with tc.tile_pool(name="persist", bufs=1) as pers:
            in1Ts = pers.tile([P, KI, B, S], bf16, name="in1Ts")
            in2Ts = pers.tile([P, KI, B, S], bf16, name="in2Ts")
            tBs = pers.tile([P, B, YB, OPC], f32, name="tBs")

            with tc.tile_pool(name="w1p", bufs=2 * OC) as w1p, \
                 tc.tile_pool(name="tempp", bufs=temp_bufs) as tempp, \
                 tc.tile_pool(name="repp", bufs=2 * B) as repp, \
                 tc.tile_pool(name="otp", bufs=3) as otp, \
                 tc.tile_pool(name="psp", bufs=4, space="PSUM") as psp:
                def stream_w1_o(c, oo):
                    t = w1p.tile([P, KI, IN], bf16, name="w1t", tag="w1t")
                    nc.sync.dma_start(
                        t, w1[:, c * OC + oo, :].rearrange("(a p) j -> p a j", p=P))
                    return t

                def stream_w1(c):
                    return [stream_w1_o(c, oo) for oo in range(OC)]

                def stream_rep(c):
                    rept = []
                    for b in range(B):
                        r = repp.tile([P, OC, S], f32, name="rep", tag="rep")
                        nc.sync.dma_start(r, tArep[:, b, c * OC:(c + 1) * OC, :])
                        rept.append(r)
                    return rept

                def p1_group(c, temp, w1t, oo, jb):
                    # One [128,2,512] psum tile = 2 banks. Phase-1 groups take
                    # two tiles (all 4 batches share each weight load -> 16
                    # MMs per group, 4 per LDWEIGHTS); phase-2 groups take one
                    # (o-pair). A single 4-buf pool = 8 banks, time-shared.
                    psA = psp.tile([P, 2, S], f32, name="ps", tag="ps")
                    psB = psp.tile([P, 2, S], f32, name="ps", tag="ps")
                    for ib in range(KI):
                        lhsT = w1t[oo][:, ib, jb * P:(jb + 1) * P]
                        st = dict(start=(ib == 0), stop=(ib == KI - 1))
                        nc.tensor.matmul(psA[:, 0, :], lhsT, in1Ts[:, ib, 0, :], **st)
                        nc.tensor.matmul(psA[:, 1, :], lhsT, in1Ts[:, ib, 1, :], **st)
                        nc.tensor.matmul(psB[:, 0, :], lhsT, in1Ts[:, ib, 2, :], **st)
                        nc.tensor.matmul(psB[:, 1, :], lhsT, in1Ts[:, ib, 3, :], **st)
                    nc.scalar.activation(temp[:, jb, 0:2, oo, :], psA, COPY)
                    nc.scalar.activation(temp[:, jb, 2:4, oo, :], psB, COPY)

                def p2_group(c, temp, rept, b, yb):
                    ps = psp.tile([P, 2, S], f32, name="ps", tag="ps")
                    for jb in range(KI):
                        lhsT = in2Ts[:, jb, b, yb * P:(yb + 1) * P]
                        for oo in range(OC):
                            nc.tensor.matmul(
                                ps[:, oo, :], lhsT, temp[:, jb, b, oo, :],
                                start=(jb == 0), stop=(jb == KI - 1))
                    ot = otp.tile([P, OC, S], f32, name="ot", tag="ot")
                    for oo in range(OC):
                        # out = (psum + termB[y,o]) + termA_rep[o, x]
                        o = c * OC + oo
                        nc.vector.scalar_tensor_tensor(
                            ot[:, oo, :], ps[:, oo, :],
                            tBs[:, b, yb, o:o + 1],
                            rept[b][:, oo, :], ADD, ADD)
                    nc.sync.dma_start(
                        outp[b, yb * P:(yb + 1) * P, c * OC:(c + 1) * OC, :], ot)

                # DMA order at startup: chunk-0 w1 + in1T first (phase 1's
                # only inputs; the sim serializes concurrent DMA transfers,
                # so big phase-2-only loads must not delay them), everything
                # phase-2 related after phase 1 of chunk 0 is emitted.
                #
                # Emission interleaves phase 2 of chunk c-1 with phase 1 of
                # chunk c (2 P2 groups per P1 group, both ~27.3us per chunk)
                # so the PE instruction stream never breaks at a phase
                # boundary -- an idle PE also resets the clock p-state, which
                # costs ~3us of half-speed ramp per gap on top of the gap.
                # in1T arrives in 4 per-ib pieces interleaved with the two w1
                # tiles so chunk-0 phase 1 starts as soon as (w1[o0], ib0)
                # land instead of after one monolithic 6us load
                w1t_cur = []
                w1t_cur.append(stream_w1_o(0, 0))
                for ib in range(KI):
                    nc.sync.dma_start(
                        in1Ts[:, ib, :, :],
                        in1T[ib * P:(ib + 1) * P, :, :].rearrange(
                            "(a p) b x -> p (a b) x", p=P))
                    if ib == 0:
                        w1t_cur.append(stream_w1_o(0, 1))
                temp_cur = tempp.tile([P, KI, B, OC, S], bf16, name="temp", tag="temp")
                for oo in range(OC) if only_phase in (0, 1) else []:
                    for jb in range(KI):
                        p1_group(0, temp_cur, w1t_cur, oo, jb)
                if only_phase == 1:
                    for c in range(1, NCH):
                        w1t_cur = stream_w1(c)
                        temp_cur = tempp.tile([P, KI, B, OC, S], bf16,
                                              name="temp", tag="temp")
                        for jb in range(KI):
                            for oo in range(OC):
                                p1_group(c, temp_cur, w1t_cur, oo, jb)
                if only_phase == 0:
                    nc.sync.dma_start(
                        in2Ts, in2T.rearrange("(a p) b y -> p a b y", p=P))
                    nc.sync.dma_start(tBs, tBbT[:, :, :, :])
                    rep_cur = stream_rep(0)
                    w1t_nxt = stream_w1(1)
                    for c in range(1, NCH):
                        # prefetch emitted a full block (~55us) ahead of use
                        w1t, w1t_nxt = w1t_nxt, (stream_w1(c + 1)
                                                 if c + 1 < NCH else None)
                        rep_nxt = stream_rep(c)
                        temp_nxt = tempp.tile([P, KI, B, OC, S], bf16,
                                              name="temp", tag="temp")
                        p2s = [(b, yb) for b in range(B) for yb in range(YB)]
                        p1s = [(oo, jb) for jb in range(KI) for oo in range(OC)]
                        for k in range(8):
                            p2_group(c - 1, temp_cur, rep_cur, *p2s[2 * k])
                            p2_group(c - 1, temp_cur, rep_cur, *p2s[2 * k + 1])
                            p1_group(c, temp_nxt, w1t, *p1s[k])
                        temp_cur, rep_cur = temp_nxt, rep_nxt
                    for b in range(B):
                        for yb in range(YB):
                            p2_group(NCH - 1, temp_cur, rep_cur, b, yb)

    if split_waits:
        split_sync_waits(nc)
    return nc


_CACHE = {}


def _get_nc(**kw):
    key = tuple(sorted(kw.items()))
    if key not in _CACHE:
        _CACHE[key] = build_nc(**kw)
    return _CACHE[key]


TRACE = False
LAST_RESULT = None


def kernel(input1, input2, w1, w2, seq_len=None, **_ignored):
    global LAST_RESULT
    from concourse.bass_utils import run_bass_kernel_spmd
    import ml_dtypes

    bf16 = ml_dtypes.bfloat16
    input1 = np.asarray(input1, dtype=np.float32)
    input2 = np.asarray(input2, dtype=np.float32)
    w1 = np.asarray(w1, dtype=np.float32)
    w2 = np.asarray(w2, dtype=np.float32)

    nc = _get_nc()

    # host-side layout prep (cheap): transposed bf16 inputs, affine terms
    in1T = np.ascontiguousarray(input1.transpose(2, 0, 1)).astype(bf16)  # [IN,B,S]
    in2T = np.ascontiguousarray(input2.transpose(2, 0, 1)).astype(bf16)
    wA, wB, bias = w2[:IN], w2[IN:2 * IN], w2[2 * IN]
    termA = np.einsum('bxi,io->box', input1, wA)            # [B, OUT, S]
    termB = input1 @ wB + bias                              # [B, S, OUT]

    in_maps = []
    for c in range(N_CORES):
        o0 = c * OPC
        w1c = np.ascontiguousarray(w1[:, o0:o0 + OPC, :]).astype(bf16)
        tA = np.ascontiguousarray(termA[:, o0:o0 + OPC, :], dtype=np.float32)
        tArep = np.ascontiguousarray(
            np.broadcast_to(tA[None], (P, B, OPC, S)))      # [128,B,OPC,S]
        # tBbT[p, b, yb, o] = termB[b, yb*128+p, o0+o]
        tBbT = np.ascontiguousarray(
            termB[:, :, o0:o0 + OPC].reshape(B, S // P, P, OPC)
            .transpose(2, 0, 1, 3), dtype=np.float32)
        in_maps.append({
            "in1T": in1T,
            "in2T": in2T,
            "w1": w1c,
            "tArep": tArep,
            "tBbT": tBbT,
        })
    res = run_bass_kernel_spmd(nc, in_maps, core_ids=list(range(N_CORES)),
                               trace=TRACE)
    LAST_RESULT = res

    full = np.empty((B, S, S, OUT), dtype=np.float32)
    for c in range(N_CORES):
        o0 = c * OPC
        # device layout [b, y, o, x] -> [b, x, y, o]
        full[:, :, :, o0:o0 + OPC] = res.results[c]["outp"].transpose(0, 3, 1, 2)
    return full


# revision 34
# speedup vs baseline: 1.1993x; 1.0044x over previous
"""Biaffine (trilinear + concat-linear) kernel for Trainium2, 8-core SPMD.

logits[b,x,y,o] = sum_ij in1[b,x,i] * w1[i,o,j] * in2[b,y,j]
               + termA[b,x,o] + termB[b,y,o] + bias[o]
  termA[b,x,o] = sum_i in1[b,x,i] * w2[i,o]
  termB[b,y,o] = sum_j in1[b,y,j] * w2[IN+j,o]   (both halves from input1!)
  bias[o]      = w2[2*IN,o]

Sharding: OUTPUT-dim sharding. Core c owns o in [14c, 14c+14), ALL batches
and the full S x S plane. This cuts per-core w1 HBM traffic 8x vs
batch/x sharding (7.3 MB bf16 instead of 58.7 MB) and lets both matmul
phases batch their moving operand over the batch dim, so each stationary
(weight) load streams 1024-2048 columns instead of 256 -> far fewer
weight loads (the dominant un-modeled HW cost) and fewer instructions.

Host-side prep (cheap, O(S*IN) or O(S*OUT) only):
  - in1T/in2T: inputs pre-transposed to [IN, B, S] and cast to bf16
    (kills all on-device PE transposes of the baseline).
  - termA/termB+bias: 60 MFLOP of affine matmuls (0.025% of total work)
    computed in numpy; termA is uploaded pre-replicated across the 128
    partitions so the device can add it along the free axis with a plain
    DVE op (a partition-stride-0 read is impossible for DVE; the
    baseline burned PE columns on a "selector matmul" for this instead).

Device, per o-pair chunk (7 chunks of OC=2):
  phase 1: temp[j, b, o, x] = sum_i w1[i,o,j] * in1T[i,(b,x)]
           stationary = w1 128x128 tile (reused for 4 batch-matmuls),
           moving = in1T [128, 512], fp32 PSUM accumulate over 4
           i-blocks, drained fp32->bf16 on the ACT engine.
  phase 2: out[y, (o,x)] = sum_jb in2T-tile^T @ temp-tile
           stationary = in2T 128x128 tile (reused for 2 o-matmuls),
           moving = temp [128, 512]; drain is ONE fused DVE
           scalar_tensor_tensor per (o): out = (psum + termB[y,o]) +
           termA_rep[o,x] -- both affine terms applied in a single pass,
           nothing but matmuls ever touches the PE.
Emission interleaves phase 2 of chunk c-1 with phase 1 of chunk c (temp
double-buffered) so the PE instruction stream never breaks at a phase
boundary -- a PE idle gap also resets the clock p-state, costing ~3us
of half-speed ramp on top of the gap. All PSUM comes from one 4-buf
pool of [128,2,512] tiles (8 banks): a phase-1 group holds two tiles
(16 MMs, 4 per LDWEIGHTS), a phase-2 group one (8 MMs, 2 per LDW).
Startup streams chunk-0 w1 + per-ib in1T pieces before anything
phase-2-related so the first matmul issues ~2us in.
Device output layout [b, y, o_local, x] in bf16 (2 KB contiguous DMA
lines, halves the dominant HBM stream; output rounding adds <=0.4%
rel-to-max against a 2e-2 gate); the host upcasts and transposes to
[b, x, y, o] while unsharding. termA_rep is also bf16 (it only seeds
the fp32 affine add). Per-core HBM traffic: ~48 MB vs ~120 MB for the
batch/x-sharded baseline.
"""

import numpy as np

B, S, IN, OUT = 4, 512, 512, 112
N_CORES = 8
P = 128
OC = 2                    # o's per chunk (o-pair)
OPC = OUT // N_CORES      # 14 o's per core
NCH = OPC // OC           # 7 chunks


def split_sync_waits(nc, max_waits=1):
    """The walrus codegen in this toolchain rejects instructions carrying
    more than a few semaphore waits ("Too many sync wait commands").
    Hoist overflow waits onto NoOps inserted just before the instruction,
    on the same engine (semantically identical: the sequencer blocks on
    each wait in order)."""
    import concourse.mybir as mybir

    n_split = 0
    for f in nc.m.functions:
        for bb in f.blocks:
            new_insts = []
            for inst in bb.instructions:
                si = inst.sync_info
                if si is not None and si.on_wait and len(si.on_wait) > max_waits:
                    waits = list(si.on_wait)
                    overflow, keep = waits[:-max_waits], waits[-max_waits:]
                    for k in range(0, len(overflow), max_waits):
                        chunk = overflow[k:k + max_waits]
                        nop = mybir.InstNoOp(
                            name=f"{inst.name}_wsplit{k}",
                            opcode="NoOp",
                            engine=inst.engine,
                            sync_info=mybir.SyncInfo(on_wait=chunk, on_update=[]),
                        )
                        new_insts.append(nop)
                        n_split += 1
                    si.on_wait = keep
                new_insts.append(inst)
            bb.instructions[:] = new_insts
    return n_split


def build_nc(temp_bufs=2, split_waits=True, only_phase=0):
    """Build the per-core Bass module. All 8 cores run the same program on
    their own w1/termA/termB o-slices (SPMD)."""
    import concourse.bass as bass
    import concourse.mybir as mybir
    import concourse.tile as tile

    f32 = mybir.dt.float32
    bf16 = mybir.dt.bfloat16
    ADD = mybir.AluOpType.add
    COPY = mybir.ActivationFunctionType.Copy

    KI = IN // P   # 4 contraction blocks (i and j)
    YB = S // P    # 4 y blocks

    nc = bass.Bass()
    in1T = nc.dram_tensor("in1T", [IN, B, S], bf16, kind="ExternalInput")
    in2T = nc.dram_tensor("in2T", [IN, B, S], bf16, kind="ExternalInput")
    w1 = nc.dram_tensor("w1", [IN, OPC, IN], bf16, kind="ExternalInput")
    tArep = nc.dram_tensor("tArep", [P, B, OPC, S], bf16, kind="ExternalInput")
    tBbT = nc.dram_tensor("tBbT", [P, B, YB, OPC], f32, kind="ExternalInput")
    outp = nc.dram_tensor("outp", [B, S, OPC, S], bf16, kind="ExternalOutput")

    with tile.TileContext(nc) as tc:
        with tc.tile_pool(name="persist", bufs=1) as pers:
            in1Ts = pers.tile([P, KI, B, S], bf16, name="in1Ts")
            in2Ts = pers.tile([P, KI, B, S], bf16, name="in2Ts")
            tBs = pers.tile([P, B, YB, OPC], f32, name="tBs")

            with tc.tile_pool(name="w1p", bufs=2 * OC) as w1p, \
                 tc.tile_pool(name="tempp", bufs=temp_bufs) as tempp, \
                 tc.tile_pool(name="repp", bufs=2 * B) as repp, \
                 tc.tile_pool(name="otp", bufs=3) as otp, \
                 tc.tile_pool(name="psp", bufs=4, space="PSUM") as psp:
                def stream_w1_o(c, oo):
                    t = w1p.tile([P, KI, IN], bf16, name="w1t", tag="w1t")
                    nc.sync.dma_start(
                        t, w1[:, c * OC + oo, :].rearrange("(a p) j -> p a j", p=P))
                    return t

                def stream_w1(c):
                    return [stream_w1_o(c, oo) for oo in range(OC)]

                def stream_rep(c):
                    rept = []
                    for b in range(B):
                        r = repp.tile([P, OC, S], bf16, name="rep", tag="rep")
                        nc.sync.dma_start(r, tArep[:, b, c * OC:(c + 1) * OC, :])
                        rept.append(r)
                    return rept

                def p1_group(c, temp, w1t, oo, jb):
                    # One [128,2,512] psum tile = 2 banks. Phase-1 groups take
                    # two tiles (all 4 batches share each weight load -> 16
                    # MMs per group, 4 per LDWEIGHTS); phase-2 groups take one
                    # (o-pair). A single 4-buf pool = 8 banks, time-shared.
                    psA = psp.tile([P, 2, S], f32, name="ps", tag="ps")
                    psB = psp.tile([P, 2, S], f32, name="ps", tag="ps")
                    for ib in range(KI):
                        lhsT = w1t[oo][:, ib, jb * P:(jb + 1) * P]
                        st = dict(start=(ib == 0), stop=(ib == KI - 1))
                        nc.tensor.matmul(psA[:, 0, :], lhsT, in1Ts[:, ib, 0, :], **st)
                        nc.tensor.matmul(psA[:, 1, :], lhsT, in1Ts[:, ib, 1, :], **st)
                        nc.tensor.matmul(psB[:, 0, :], lhsT, in1Ts[:, ib, 2, :], **st)
                        nc.tensor.matmul(psB[:, 1, :], lhsT, in1Ts[:, ib, 3, :], **st)
                    nc.scalar.activation(temp[:, jb, 0:2, oo, :], psA, COPY)
                    nc.scalar.activation(temp[:, jb, 2:4, oo, :], psB, COPY)

                def p2_group(c, temp, rept, b, yb):
                    ps = psp.tile([P, 2, S], f32, name="ps", tag="ps")
                    for jb in range(KI):
                        lhsT = in2Ts[:, jb, b, yb * P:(yb + 1) * P]
                        for oo in range(OC):
                            nc.tensor.matmul(
                                ps[:, oo, :], lhsT, temp[:, jb, b, oo, :],
                                start=(jb == 0), stop=(jb == KI - 1))
                    ot = otp.tile([P, OC, S], bf16, name="ot", tag="ot")
                    for oo in range(OC):
                        # out = (psum + termB[y,o]) + termA_rep[o, x]
                        o = c * OC + oo
                        nc.vector.scalar_tensor_tensor(
                            ot[:, oo, :], ps[:, oo, :],
                            tBs[:, b, yb, o:o + 1],
                            rept[b][:, oo, :], ADD, ADD)
                    nc.sync.dma_start(
                        outp[b, yb * P:(yb + 1) * P, c * OC:(c + 1) * OC, :], ot)

                # DMA order at startup: chunk-0 w1 + in1T first (phase 1's
                # only inputs; the sim serializes concurrent DMA transfers,
                # so big phase-2-only loads must not delay them), everything
                # phase-2 related after phase 1 of chunk 0 is emitted.
                #
                # Emission interleaves phase 2 of chunk c-1 with phase 1 of
                # chunk c (2 P2 groups per P1 group, both ~27.3us per chunk)
                # so the PE instruction stream never breaks at a phase
                # boundary -- an idle PE also resets the clock p-state, which
                # costs ~3us of half-speed ramp per gap on top of the gap.
                # in1T arrives in 4 per-ib pieces interleaved with the two w1
                # tiles so chunk-0 phase 1 starts as soon as (w1[o0], ib0)
                # land instead of after one monolithic 6us load
                w1t_cur = []
                w1t_cur.append(stream_w1_o(0, 0))
                for ib in range(KI):
                    nc.sync.dma_start(
                        in1Ts[:, ib, :, :],
                        in1T[ib * P:(ib + 1) * P, :, :].rearrange(
                            "(a p) b x -> p (a b) x", p=P))
                    if ib == 0:
                        w1t_cur.append(stream_w1_o(0, 1))
                temp_cur = tempp.tile([P, KI, B, OC, S], bf16, name="temp", tag="temp")
                for oo in range(OC) if only_phase in (0, 1) else []:
                    for jb in range(KI):
                        p1_group(0, temp_cur, w1t_cur, oo, jb)
                if only_phase == 1:
                    for c in range(1, NCH):
                        w1t_cur = stream_w1(c)
                        temp_cur = tempp.tile([P, KI, B, OC, S], bf16,
                                              name="temp", tag="temp")
                        for jb in range(KI):
                            for oo in range(OC):
                                p1_group(c, temp_cur, w1t_cur, oo, jb)
                if only_phase == 0:
                    nc.sync.dma_start(
                        in2Ts, in2T.rearrange("(a p) b y -> p a b y", p=P))
                    nc.sync.dma_start(tBs, tBbT[:, :, :, :])
                    rep_cur = stream_rep(0)
                    w1t_nxt = stream_w1(1)
                    for c in range(1, NCH):
                        # prefetch emitted a full block (~55us) ahead of use
                        w1t, w1t_nxt = w1t_nxt, (stream_w1(c + 1)
                                                 if c + 1 < NCH else None)
                        rep_nxt = stream_rep(c)
                        temp_nxt = tempp.tile([P, KI, B, OC, S], bf16,
                                              name="temp", tag="temp")
                        p2s = [(b, yb) for b in range(B) for yb in range(YB)]
                        p1s = [(oo, jb) for jb in range(KI) for oo in range(OC)]
                        for k in range(8):
                            p2_group(c - 1, temp_cur, rep_cur, *p2s[2 * k])
                            p2_group(c - 1, temp_cur, rep_cur, *p2s[2 * k + 1])
                            p1_group(c, temp_nxt, w1t, *p1s[k])
                        temp_cur, rep_cur = temp_nxt, rep_nxt
                    for b in range(B):
                        for yb in range(YB):
                            p2_group(NCH - 1, temp_cur, rep_cur, b, yb)

    if split_waits:
        split_sync_waits(nc)
    return nc


_CACHE = {}


def _get_nc(**kw):
    key = tuple(sorted(kw.items()))
    if key not in _CACHE:
        _CACHE[key] = build_nc(**kw)
    return _CACHE[key]


TRACE = False
LAST_RESULT = None


def kernel(input1, input2, w1, w2, seq_len=None, **_ignored):
    global LAST_RESULT
    from concourse.bass_utils import run_bass_kernel_spmd
    import ml_dtypes

    bf16 = ml_dtypes.bfloat16
    input1 = np.asarray(input1, dtype=np.float32)
    input2 = np.asarray(input2, dtype=np.float32)
    w1 = np.asarray(w1, dtype=np.float32)
    w2 = np.asarray(w2, dtype=np.float32)

    nc = _get_nc()

    # host-side layout prep (cheap): transposed bf16 inputs, affine terms
    in1T = np.ascontiguousarray(input1.transpose(2, 0, 1)).astype(bf16)  # [IN,B,S]
    in2T = np.ascontiguousarray(input2.transpose(2, 0, 1)).astype(bf16)
    wA, wB, bias = w2[:IN], w2[IN:2 * IN], w2[2 * IN]
    termA = np.einsum('bxi,io->box', input1, wA)            # [B, OUT, S]
    termB = input1 @ wB + bias                              # [B, S, OUT]

    in_maps = []
    for c in range(N_CORES):
        o0 = c * OPC
        w1c = np.ascontiguousarray(w1[:, o0:o0 + OPC, :]).astype(bf16)
        tA = termA[:, o0:o0 + OPC, :].astype(bf16)
        tArep = np.ascontiguousarray(
            np.broadcast_to(tA[None], (P, B, OPC, S)))      # [128,B,OPC,S]
        # tBbT[p, b, yb, o] = termB[b, yb*128+p, o0+o]
        tBbT = np.ascontiguousarray(
            termB[:, :, o0:o0 + OPC].reshape(B, S // P, P, OPC)
            .transpose(2, 0, 1, 3), dtype=np.float32)
        in_maps.append({
            "in1T": in1T,
            "in2T": in2T,
            "w1": w1c,
            "tArep": tArep,
            "tBbT": tBbT,
        })
    res = run_bass_kernel_spmd(nc, in_maps, core_ids=list(range(N_CORES)),
                               trace=TRACE)
    LAST_RESULT = res

    full = np.empty((B, S, S, OUT), dtype=np.float32)
    for c in range(N_CORES):
        o0 = c * OPC
        # device layout [b, y, o, x] (bf16) -> [b, x, y, o] fp32
        full[:, :, :, o0:o0 + OPC] = (
            res.results[c]["outp"].astype(np.float32).transpose(0, 3, 1, 2))
    return full
